# revision 7
# baseline (speedup 1.0000x reference)
"""Llama GQA causal attention (S=2048, D=4096, 32 q-heads / 8 kv-heads,
head_dim=128) on 8 Trainium2 NeuronCores.

Sharding: tensor-parallel over heads. Core c owns q-heads [4c, 4c+4) and
kv-head c. Each core computes its QKV slice from the full hidden_states,
runs causal attention for its 4 q-heads, and produces a partial
o-projection y_c = attn_out_c @ Wo[512c:512c+512, :] (bf16). The host
sums the 8 partials.

v4 design notes (on top of v3):
  - Host pre-casts/pre-transposes all inputs into SBUF-ready layouts.
  - Q/K projections in fp8e4m3 DoubleRow (two d-blocks per matmul);
    descale folded into the PSUM->SBUF copy. V/attention/o-proj bf16.
  - scoresT[k,(h,q)] layout softmax: no max pass, no probs transposes;
    denominator from an all-ones lhsT matmul (result arrives partition-
    replicated), 1/l folded into the oT writeback mul.
  - The o-projection for row-block i-1 is interleaved right after
    attention(i): y stores spread across the kernel, no phase-C tail,
    and single-buffered PSUM accumulators (ops/lps) drain during the
    interleaved o-proj matmuls.
  - DMA issue order tuned so the PE chases the first quarter-DMAs of
    w8/x8 at startup; wo loads sit behind the chunk-0 critical loads.
"""

import sys

if "/opt/trn_rl_repo" not in sys.path:
    sys.path.insert(0, "/opt/trn_rl_repo")

import numpy as np

S = 2048
D = 4096
HD = 128
G = 4            # q heads per core
NCORES = 8
NB = S // 128    # 16 s-blocks
DB = D // 128    # 32 d-blocks
DB2 = DB // 2    # 16 d-block pairs (DoubleRow)
SCH = 4          # s-chunks of 512
WCOLS = G * HD + 2 * HD  # 768 qkv cols per core
QK = 5 * HD      # 640 fp8 (q+k) cols per core
FP8_SCALE = 256.0

_cache = {}


def _build():
    import concourse.bacc as bacc
    import concourse.mybir as mybir
    from concourse import tile

    f32 = mybir.dt.float32
    bf16 = mybir.dt.bfloat16
    f8 = mybir.dt.float8e4
    EXP = mybir.ActivationFunctionType.Exp
    MUL = mybir.AluOpType.mult
    DR = mybir.MatmulPerfMode.DoubleRow

    nc = bacc.Bacc(None, target_bir_lowering=False, debug=False)
    # host-prepped layouts (see _shard_inputs)
    xt_d = nc.declare_dram_parameter("xt", [SCH, 128, DB, 512], bf16, isOutput=False)
    x8_d = nc.declare_dram_parameter("x8", [SCH, 128, DB2, 2, 512], f8, isOutput=False)
    w8_d = nc.declare_dram_parameter("w8", [128, DB2, 2, QK], f8, isOutput=False)
    wv_d = nc.declare_dram_parameter("wv", [128, DB, HD], bf16, isOutput=False)
    wo_d = nc.declare_dram_parameter("wo", [128, G, D], bf16, isOutput=False)
    y_d = nc.declare_dram_parameter("y", [NB, 128, D], bf16, isOutput=True)

    qdescale = float(HD ** -0.5 / (FP8_SCALE * FP8_SCALE))
    kdescale = float(1.0 / (FP8_SCALE * FP8_SCALE))

    with tile.TileContext(nc) as tc:
        with (
            tc.tile_pool(name="persist", bufs=1) as pp,
            tc.tile_pool(name="expp", bufs=4) as pe,
            tc.tile_pool(name="wop", bufs=1) as pw,
            tc.tile_pool(name="xtp", bufs=1) as px,
            tc.tile_pool(name="x8p", bufs=1) as px8,
            tc.tile_pool(name="linvp", bufs=1) as pl,
            tc.tile_pool(name="yp", bufs=2) as pyb,
            tc.tile_pool(name="ps512", bufs=4, space="PSUM") as ps_a,
            tc.tile_pool(name="ps_o", bufs=1, space="PSUM") as ps_o,
            tc.tile_pool(name="ps_l", bufs=1, space="PSUM") as ps_l,
            tc.tile_pool(name="ps_y", bufs=2, space="PSUM") as ps_y,
        ):
            qkvT = pp.tile([128, 6, S], bf16)    # [:, 0:4, :] qT; [:, 4, :] kT; [:, 5, :] vT
            v_nat = pp.tile([128, NB, HD], bf16)  # block t: [k-local, dh]
            oT = pp.tile([128, NB, G, 128], bf16)  # block i: [dh, h, q]
            w8_sb = pp.tile([128, DB2, 2, QK], f8)
            wv_sb = pp.tile([128, DB, HD], bf16)
            ones = pp.tile([128, 128], bf16)
            cmT = pp.tile([128, 512], f32)       # 4x tiled upper-tri -30000 mask
            wo_sb = pw.tile([128, G, D], bf16)

            nc.vector.memset(ones[:], 1.0)
            nc.gpsimd.memset(cmT[:], 0.0)
            for h in range(G):
                # cmT[k, h*128+q] = (q - k) >= 0 ? 0 : -30000
                nc.gpsimd.affine_select(
                    out=cmT[:, h * 128:(h + 1) * 128],
                    in_=cmT[:, h * 128:(h + 1) * 128],
                    compare_op=mybir.AluOpType.is_ge,
                    fill=-30000.0,
                    base=0,
                    pattern=[[1, 128]],
                    channel_multiplier=-1,
                )

            x8_tiles = {}

            def load_x8(sc):
                x8 = px8.tile([128, DB2, 2, 512], f8, tag="x8")
                for q4 in range(4):
                    nc.sync.dma_start(
                        x8[:, q4 * 4:(q4 + 1) * 4, :, :],
                        x8_d[sc, :, q4 * 4:(q4 + 1) * 4, :, :],
                    )
                x8_tiles[sc] = x8

            xt_tiles = {}

            def load_xt(sc):
                xT = px.tile([128, DB, 512], bf16, tag="xT")
                for q4 in range(4):
                    nc.sync.dma_start(
                        xT[:, q4 * 8:(q4 + 1) * 8, :],
                        xt_d[sc, :, q4 * 8:(q4 + 1) * 8, :],
                    )
                xt_tiles[sc] = xT

            # chunk-0 critical loads, quarter-interleaved so the PE can
            # chase the first arrivals
            x8_0 = px8.tile([128, DB2, 2, 512], f8, tag="x8")
            for q4 in range(4):
                nc.sync.dma_start(
                    x8_0[:, q4 * 4:(q4 + 1) * 4, :, :],
                    x8_d[0, :, q4 * 4:(q4 + 1) * 4, :, :],
                )
                nc.sync.dma_start(
                    w8_sb[:, q4 * 4:(q4 + 1) * 4, :, :],
                    w8_d[:, q4 * 4:(q4 + 1) * 4, :, :],
                )
            x8_tiles[0] = x8_0
            load_xt(0)
            for q4 in range(4):
                nc.sync.dma_start(
                    wv_sb[:, q4 * 8:(q4 + 1) * 8, :],
                    wv_d[:, q4 * 8:(q4 + 1) * 8, :],
                )
            # o-proj weights: first needed ~40us in; 8 spread DMAs
            for hb in range(G):
                for half in range(2):
                    nc.sync.dma_start(
                        wo_sb[:, hb, half * 2048:(half + 1) * 2048],
                        wo_d[:, hb, half * 2048:(half + 1) * 2048],
                    )

            def emit_oproj(i):
                y_sb = pyb.tile([128, D], bf16, tag="y_sb")
                for n in range(8):
                    py = ps_y.tile([128, 512], f32, tag="yps")
                    for hb in range(G):
                        nc.tensor.matmul(
                            py[:],
                            oT[:, i, hb, :],
                            wo_sb[:, hb, n * 512:(n + 1) * 512],
                            start=(hb == 0),
                            stop=(hb == G - 1),
                        )
                    if n % 2 == 0:
                        nc.vector.tensor_copy(y_sb[:, n * 512:(n + 1) * 512], py[:])
                    else:
                        nc.scalar.copy(y_sb[:, n * 512:(n + 1) * 512], py[:])
                nc.sync.dma_start(y_d[i], y_sb[:])

            for sc in range(SCH):
                xT = xt_tiles.pop(sc)
                x8 = x8_tiles.pop(sc)

                # ---- Q/K for this chunk: fp8 DoubleRow ----
                for cb in range(5):
                    pm = ps_a.tile([128, 512], f32, tag="s512")
                    for db2 in range(DB2):
                        nc.tensor.matmul(
                            pm[:],
                            w8_sb[:, db2, :, cb * 128:(cb + 1) * 128],
                            x8[:, db2, :, :],
                            start=(db2 == 0),
                            stop=(db2 == DB2 - 1),
                            perf_mode=DR,
                        )
                    nc.scalar.mul(
                        qkvT[:, cb, sc * 512:(sc + 1) * 512], pm[:],
                        qdescale if cb < 4 else kdescale,
                    )
                if sc + 1 < SCH:
                    load_x8(sc + 1)   # single buffer: reload after Q/K consumed it
                # ---- V for this chunk: bf16 ----
                pm = ps_a.tile([128, 512], f32, tag="s512")
                for db in range(DB):
                    nc.tensor.matmul(
                        pm[:],
                        wv_sb[:, db, :],
                        xT[:, db, :],
                        start=(db == 0),
                        stop=(db == DB - 1),
                    )
                nc.scalar.copy(qkvT[:, 5, sc * 512:(sc + 1) * 512], pm[:])
                if sc + 1 < SCH:
                    load_xt(sc + 1)   # single buffer: reload after V consumed it
                # ---- v natural for this chunk (XBAR transpose) ----
                nc.sync.dma_start_transpose(
                    v_nat[:, sc * 4:(sc + 1) * 4, :],
                    qkvT[:, 5, sc * 512:(sc + 1) * 512],
                )

                # ---- causal attention + lagged o-projection ----
                for i in range(sc * 4, sc * 4 + 4):
                    qT4 = qkvT[:, 0:G, i * 128:(i + 1) * 128]  # [128, 4, 128]
                    sps = {}

                    def emit_scores(t):
                        sp = ps_a.tile([128, 512], f32, tag="s512")
                        nc.tensor.matmul(
                            sp[:],
                            qkvT[:, 4, t * 128:(t + 1) * 128],
                            qT4,
                            start=True,
                            stop=True,
                        )
                        if t == i:
                            nc.vector.tensor_add(sp[:], sp[:], cmT[:])
                        sps[t] = sp

                    emit_scores(0)
                    if i > 0:
                        emit_scores(1)
                    op = ps_o.tile([128, 512], f32, tag="ops")
                    lp = ps_l.tile([128, 512], f32, tag="lps")
                    for t in range(i + 1):
                        if t + 2 <= i:
                            emit_scores(t + 2)
                        ex = pe.tile([128, 512], bf16, tag="expT")
                        nc.scalar.activation(ex[:], sps.pop(t)[:], EXP)
                        nc.tensor.matmul(
                            op[:], v_nat[:, t, :], ex[:],
                            start=(t == 0), stop=(t == i),
                        )
                        nc.tensor.matmul(
                            lp[:], ones[:], ex[:],
                            start=(t == 0), stop=(t == i),
                        )
                    linv = pl.tile([128, 512], f32, tag="linv")
                    nc.vector.reciprocal(linv[:], lp[:])
                    nc.vector.tensor_tensor(
                        oT[:, i, :, :], op[:], linv[:], MUL
                    )
                    if i > 0:
                        emit_oproj(i - 1)

            emit_oproj(NB - 1)

    nc.finalize()
    return nc


def _get_nc():
    if "nc" not in _cache:
        _cache["nc"] = _build()
    return _cache["nc"]


def _shard_inputs(hidden_states, Wqkv, Wo):
    import ml_dtypes

    bf16 = ml_dtypes.bfloat16
    fp8 = ml_dtypes.float8_e4m3
    # x pre-transposed into [sc, p, db, s'] = x[sc*512+s', db*128+p]
    x = np.asarray(hidden_states, dtype=np.float32)
    xt_t = x.reshape(SCH, 512, DB, 128).transpose(0, 3, 2, 1)
    xt = np.ascontiguousarray(xt_t.astype(bf16))
    # fp8 copy, scaled, with d-blocks paired: [sc, p, db2, j, s']
    x8 = np.ascontiguousarray(
        (xt_t * FP8_SCALE).reshape(SCH, 128, DB2, 2, 512).astype(fp8)
    )
    q_sz = 32 * HD  # 4096
    in_maps = []
    for c in range(NCORES):
        wq = Wqkv[:, c * G * HD:(c + 1) * G * HD]
        wk = Wqkv[:, q_sz + c * HD: q_sz + (c + 1) * HD]
        wv = Wqkv[:, q_sz + 8 * HD + c * HD: q_sz + 8 * HD + (c + 1) * HD]
        # q+k cols in fp8 (x256), paired d-blocks: [p, db2, j, c]
        wqk = np.concatenate([wq, wk], axis=1).astype(np.float32) * FP8_SCALE
        w8 = np.ascontiguousarray(
            wqk.reshape(DB2, 2, 128, QK).transpose(2, 0, 1, 3).astype(fp8)
        )
        wv_c = np.ascontiguousarray(
            np.asarray(wv, dtype=np.float32)
            .reshape(DB, 128, HD).transpose(1, 0, 2).astype(bf16)
        )
        wo_c = Wo[c * G * HD:(c + 1) * G * HD, :].astype(np.float32)
        wo_c = np.ascontiguousarray(
            wo_c.reshape(G, 128, D).transpose(1, 0, 2).astype(bf16)
        )
        in_maps.append(
            {"xt": xt, "x8": x8, "w8": w8, "wv": wv_c, "wo": wo_c}
        )
    return in_maps


def run(inputs, trace=False, trace_kwargs=None):
    from concourse.bass_utils import run_bass_kernel_spmd

    if trace:
        _install_profile_hook()
    nc = _get_nc()
    in_maps = _shard_inputs(
        np.asarray(inputs["hidden_states"]),
        np.asarray(inputs["Wqkv"]),
        np.asarray(inputs["Wo"]),
    )
    res = run_bass_kernel_spmd(
        nc, in_maps, core_ids=list(range(NCORES)), trace=trace,
        **(trace_kwargs or {}),
    )
    y = np.zeros((S, D), dtype=np.float32)
    for c in range(NCORES):
        y += res.results[c]["y"].reshape(S, D).astype(np.float32)
    return y[None], res


def _install_profile_hook():
    """trn_boot couldn't register the NTFF hook (antenv.axon_hooks missing
    in this image); provide the module and register it ourselves."""
    import types

    if "antenv.axon_hooks" in sys.modules:
        return
    import antenv

    holder = [None]
    mod = types.ModuleType("antenv.axon_hooks")
    mod.set_axon_ntff_profile_hook = lambda h: holder.__setitem__(0, h)
    mod.get_axon_ntff_profile_hook = lambda: holder[0]
    sys.modules["antenv.axon_hooks"] = mod
    antenv.axon_hooks = mod
    from trn_agent_boot.trn_boot import _ntff_profile_via_ctypes

    mod.set_axon_ntff_profile_hook(
        _ntff_profile_via_ctypes("/opt/axon/libaxon_pjrt.so")
    )


def kernel(**inputs):
    out, _ = run(inputs, trace=False)
    return out
